# revision 24
# baseline (speedup 1.0000x reference)
"""AttentionNet kernel for Trainium2: 8-core data-parallel over batch.

Reference computation (per batch element b):
  emb    = x.reshape(N,64) @ conv_w + conv_b          [N,512]
  x_real = emb * mask[:,None]
  query  = sum_n(x_real) / (sum(mask)+1e-5)           [512]
  q_proj = query @ Uq                                 [512]
  r_proj = x_real @ Ur                                [N,512]
  logits = tanh(q_proj + r_proj) @ Ua                 [N]
  attn   = softmax(logits masked)                     [N]
  out    = attn @ x_real                              [512]

Kernel restructure (v4: masked-column packing, fp16 data path):
  * Masked positions contribute nothing (x_real = 0 there), and the output
    is permutation-invariant in n.  Host packs each batch's VALID columns
    first (the rest are zero), sorts all B batches by valid count, and
    groups them into tiles of TB in {8,4} batches padded to the tile max
    width w.  Tile plan is GLOBAL (shared by all 8 cores; core k takes the
    k-th contiguous TB-slice of each sorted 64/32-batch chunk), so one
    compiled kernel serves all cores.  Columns drop ~2x vs dense N=128.
  * conv fold (host): wura = [W;b] @ Ur, wauq = [W;b] @ Uq, so
    r_proj = xm @ wura[0:64] + mask*wura[64]; q_proj enters the same
    matmul through per-tile indicator rows (rank-TB update), so
    z = r_proj + q_proj is ONE K=73 matmul per (k-chunk, tile).  The
    indicator value is 1/(sum(mask)+1e-5) (host-known), which applies the
    query normalization inside the matmul -- no on-device reciprocal.
  * xaT[73, V] rows 0:64 = xmT (contiguous DMAs), row 64 = mask,
    rows 65:73 = indicators.  No on-device transposes.
  * xasum = grouped DVE reduce of xaT per tile (row 64 = sum(mask) free);
    unnormalized q_proj computed per 32-batch unit, just-in-time, so the
    main loop starts as soon as the first columns + reduces land.
  * logits = Ua_rep.T @ tanh(z) with Ua replicated across 128 cols ->
    logits replicated across partitions -> exp() broadcast is free.
    exp is batched over tile pairs (both tiles' logits share one 2-bank
    PSUM tile) to amortize activation startup.
  * unnormalized softmax: e = exp(logits-2); weighted reduce of xaT rows
    0:65 by e gives esum and Z = sum(e*mask) (row 64); masked/padded
    columns of xaT are zero so they drop out automatically.
  * out = (esum.T @ [W;b]) * (1/Z)  (fp32 finish); host un-permutes rows.
"""

import os
import sys

sys.path.insert(0, "/opt/trn_rl_repo")

import numpy as np
from contextlib import ExitStack

import concourse.bass as bass
import concourse.bacc as bacc
import concourse.tile as tile
from concourse import mybir

B, N, DOBJ, DM = 2048, 128, 64, 512
NCORES = 8
BSH = B // NCORES          # 256 batch per core
KC = 4                     # 512 = 4 chunks of 128 along d_model
NIND = 8                   # indicator rows (max TB)
KTOT = DOBJ + 1 + NIND     # 73 contraction rows
NQP = 8                    # 32-batch q_proj units
F32 = mybir.dt.float32
F16 = mybir.dt.float16
AF = mybir.ActivationFunctionType
ALU = mybir.AluOpType
AX = mybir.AxisListType
EXP_SHIFT = -2.0           # exp(logits+shift): keeps e in fp16 range


def make_plan(c):
    """Global tile plan from per-batch valid counts c[B] (any core order).

    Returns (order, plan, V): order = batches sorted by count desc;
    plan = list of (TB, w, b0, off) shared by all cores; V = packed width.
    Each plan entry consumes 8*TB consecutive sorted batches (TB per core).
    TB=8 when w <= 64 else 4, so R2 = TB*w <= 512 (one PSUM bank fp32).
    """
    order = np.argsort(-c, kind="stable")
    plan = []
    p, b0, off = 0, 0, 0
    while p < B:
        w = max(int(c[order[p]]), 1)
        TB = 8 if w <= 64 else 4
        if p + 8 * TB > B:
            TB = 4
        plan.append((TB, w, b0, off))
        p += 8 * TB
        b0 += TB
        off += TB * w
    return order, plan, off


def build_nc(plan, V):
    nc = bacc.Bacc("TRN2", target_bir_lowering=False, debug=False, num_devices=1)

    xmt = nc.dram_tensor("xmt", [DOBJ, V], F16, kind="ExternalInput")
    mask = nc.dram_tensor("mask", [1, V], F16, kind="ExternalInput")
    ind = nc.dram_tensor("ind8", [NIND, V], F16, kind="ExternalInput")
    wb32 = nc.dram_tensor("wb32", [65, DM], F32, kind="ExternalInput")
    wura16 = nc.dram_tensor("wura16", [65, DM], F16, kind="ExternalInput")
    wauq16 = nc.dram_tensor("wauq16", [65, DM], F16, kind="ExternalInput")
    uarep16 = nc.dram_tensor("uarep16", [128, DM], F16, kind="ExternalInput")
    out = nc.dram_tensor("out", [BSH, DM], F32, kind="ExternalOutput")

    # persistent SBUF
    xaT = nc.alloc_sbuf_tensor("xaT", [KTOT, V], F16).ap()
    wb = nc.alloc_sbuf_tensor("wb", [65, DM], F32).ap()            # [[W];[b]]
    wauq = nc.alloc_sbuf_tensor("wauq", [65, DM], F16).ap()
    uarep = nc.alloc_sbuf_tensor("uarep", [128, DM], F16).ap()
    recipz = nc.alloc_sbuf_tensor("recipz", [128, 2], F32).ap()
    qpt = [nc.alloc_sbuf_tensor(f"qpt{i}", [128, DM], F16).ap()
           for i in range(2)]                                      # [b, k]
    xasum = nc.alloc_sbuf_tensor("xasum", [65, BSH], F16).ap()
    xaesum = nc.alloc_sbuf_tensor("xaesum", [65, BSH], F32).ap()
    rpw = nc.alloc_sbuf_tensor("rpw", [KTOT, 4 * DM], F16).ap()    # lhsT ring
    eshift = nc.alloc_sbuf_tensor("eshift", [128, 1], F32).ap()
    scratch = nc.alloc_sbuf_tensor("scratch", [1, 1], F16).ap()

    # q_proj unit bookkeeping (PSUM AP base partition must be 0/32/64, so
    # units per 128-block are [32, 32, 64] batches); qu_tiles[u] = tiles
    # whose batches intersect the unit's range
    qu_ranges = []
    for blk in range(2):
        qu_ranges += [(blk * 128, 32), (blk * 128 + 32, 32),
                      (blk * 128 + 64, 64)]
    NQU = len(qu_ranges)
    qu_tiles = [[] for _ in range(NQU)]
    for ti, (TB, w, b0, off) in enumerate(plan):
        for u, (bs, bn) in enumerate(qu_ranges):
            if b0 < bs + bn and b0 + TB > bs:
                qu_tiles[u].append(ti)
    need = [min(qu_tiles[u]) for u in range(NQU)]
    t_end = [plan[ti][3] + plan[ti][0] * plan[ti][1] for ti in range(len(plan))]
    qu_end = [max(t_end[ti] for ti in qu_tiles[u]) for u in range(NQU)]

    with tile.TileContext(nc) as tc:
        # ---------------- setup: loads only (weights folded on host) ------
        nc.vector.memset(eshift, EXP_SHIFT)
        # TB=4 tiles leave rpw rows 69:73 unwritten; clear once so the
        # zero-indicator columns multiply against 0, not uninitialized NaNs
        # (start at partition 64 for alignment; row 64 is rewritten below)
        nc.vector.memset(rpw[64:65 + NIND, :], 0.0)

        # DMA plan: sync + scalar are the HW-DGE queues (gpsimd is slow
        # software-DGE, kept for the small per-tile q_proj row loads so the
        # HW queues stay FIFO-clean).  Queues are FIFO: order transfers by
        # when the pipeline needs them -- earliest tiles' columns first.
        def xmt_chunk(eng, c0, c1):
            if c0 < c1:
                eng.dma_start(
                    out=xaT[0:64, c0:c1],
                    in_=bass.AP(tensor=xmt, offset=c0,
                                ap=[[V, DOBJ], [1, c1 - c0]]),
                )

        def ind_chunk(eng, c0, c1):
            if c0 < c1:
                eng.dma_start(
                    out=xaT[65:65 + NIND, c0:c1],
                    in_=bass.AP(tensor=ind, offset=c0,
                                ap=[[V, NIND], [1, c1 - c0]]),
                )

        # critical path on the (otherwise idle) sync queue: first columns,
        # then the z-matmul weight ring + wauq; per-tile q_proj row loads
        # join this queue during phase B, so keep it shallow.
        xmt_chunk(nc.sync, 0, t_end[0])
        for p in range(4):
            nc.sync.dma_start(out=rpw[0:65, p * DM:(p + 1) * DM],
                              in_=wura16.ap())
        nc.sync.dma_start(out=wauq, in_=wauq16.ap())
        # bulk loads on the scalar queue, earliest-needed first
        nc.scalar.dma_start(out=xaT[64:65, :], in_=mask.ap())
        xmt_chunk(nc.scalar, t_end[0], t_end[1])
        xmt_chunk(nc.scalar, t_end[1], t_end[3])
        xmt_chunk(nc.scalar, t_end[3], qu_end[0])
        ind_chunk(nc.scalar, 0, qu_end[0])
        nc.scalar.dma_start(out=uarep, in_=uarep16.ap())
        xmt_chunk(nc.scalar, qu_end[0], qu_end[1])
        xmt_chunk(nc.scalar, qu_end[1], qu_end[2])
        # tail transfers are emitted mid-phase-B (see late_dmas) so neither
        # queue is deep when the pipeline starts
        late_dmas = [
            (4, lambda: ind_chunk(nc.sync, qu_end[0], qu_end[1])),
            (8, lambda: xmt_chunk(nc.sync, qu_end[2], qu_end[3])),
            (12, lambda: ind_chunk(nc.sync, qu_end[1], qu_end[3])),
            (16, lambda: xmt_chunk(nc.sync, qu_end[3], qu_end[4])),
            (20, lambda: xmt_chunk(nc.sync, qu_end[4], V)),
            (22, lambda: ind_chunk(nc.sync, qu_end[3], V)),
            (26, lambda: nc.sync.dma_start(out=wb, in_=wb32.ap())),
        ]

        # ---------------- main ----------------
        with ExitStack() as ctx:
            zps = ctx.enter_context(tc.tile_pool(name="zps", bufs=2, space="PSUM"))
            lps = ctx.enter_context(tc.tile_pool(name="lps", bufs=2, space="PSUM"))
            zsb = ctx.enter_context(tc.tile_pool(name="zsb", bufs=3))
            esb = ctx.enter_context(tc.tile_pool(name="esb", bufs=6))

            next_red = [0]

            def pump_reds(upto_ti):
                # per-tile column sums (query numerator + denominator),
                # paced so they don't burst ahead of the per-tile DVE work
                while next_red[0] <= min(upto_ti, len(plan) - 1):
                    tj = next_red[0]
                    next_red[0] += 1
                    TB, w, b0, off = plan[tj]
                    with nc.allow_low_precision(reason="fp16 xasum"):
                        nc.vector.reduce_sum(
                            out=xasum[0:65, b0:b0 + TB],
                            in_=xaT[0:65, off:off + TB * w].rearrange(
                                "p (g n) -> p g n", n=w
                            ),
                            axis=AX.X,
                        )

            def emit_qproj(u):
                # unnormalized q_proj for the unit's batches (the 1/denom
                # scale rides in on the indicator rows)
                bs, bn = qu_ranges[u]
                blk, p0 = divmod(bs, 128)
                qp_ps = zps.tile([128, DM], F32, tag="z")
                nc.tensor.matmul(
                    qp_ps[p0:p0 + bn, :], xasum[:, bs:bs + bn],
                    wauq, start=True, stop=True,
                )
                nc.vector.tensor_copy(
                    out=qpt[blk][p0:p0 + bn, :], in_=qp_ps[p0:p0 + bn, :]
                )

            # phase B: attention per tile, q_proj units just-in-time.
            # Tiles are processed in pairs sharing one logits PSUM tile so
            # a single exp() serves both.
            next_u = 0
            pair = []   # [(TB, w, b0, off, R2, par)] for the open pair
            logits_ps = None

            def flush_pair():
                nonlocal pair, logits_ps
                if not pair:
                    return
                rmax = max(p[4] for p in pair)
                e_sb = esb.tile([65, len(pair) * rmax], F16, tag="e")
                if len(pair) == 2:
                    nc.scalar.activation(
                        out=e_sb.rearrange("p (t c) -> p t c", t=2),
                        in_=logits_ps.rearrange(
                            "p (t c) -> p t c", t=2)[0:65, :, 0:rmax],
                        func=AF.Exp, bias=eshift[0:65, :],
                    )
                else:
                    nc.scalar.activation(
                        out=e_sb, in_=logits_ps[0:65, 0:rmax],
                        func=AF.Exp, bias=eshift[0:65, :],
                    )
                for si, (TB, w, b0, off, R2, par) in enumerate(pair):
                    prod = esb.tile([65, R2], F16, tag="prod")
                    nc.gpsimd.tensor_tensor(
                        out=prod, in0=xaT[0:65, off:off + R2],
                        in1=e_sb[:, si * rmax:si * rmax + R2], op=ALU.mult,
                    )
                    with nc.allow_low_precision(reason="fp16 prod"):
                        nc.vector.reduce_sum(
                            out=xaesum[0:65, b0:b0 + TB],
                            in_=prod.rearrange("p (g n) -> p g n", n=w),
                            axis=AX.X,
                        )
                pair = []
                logits_ps = None

            for ti, (TB, w, b0, off) in enumerate(plan):
                R2 = TB * w
                par = ti % 4
                while late_dmas and ti >= late_dmas[0][0]:
                    late_dmas.pop(0)[1]()
                pump_reds(ti + 6)
                while next_u < NQU and (
                    ti >= max(0, need[next_u] - 2)
                    or qu_ranges[next_u][0] < b0 + TB
                ):
                    pump_reds(max(qu_tiles[next_u]))
                    emit_qproj(next_u)
                    next_u += 1
                # q_proj rows of this tile -> rpw[65:65+TB] (split at block edge)
                dst = 65
                for blk in range(2):
                    lo = max(b0, blk * 128)
                    hi = min(b0 + TB, (blk + 1) * 128)
                    if lo < hi:
                        nc.sync.dma_start(
                            out=rpw[dst:dst + (hi - lo),
                                    par * DM:(par + 1) * DM],
                            in_=qpt[blk][lo - blk * 128:hi - blk * 128, :],
                        )
                        dst += hi - lo
                if logits_ps is None:
                    logits_ps = lps.tile([128, 1024], F32, tag="lg")
                si = len(pair)
                for h in range(2):
                    z_ps = zps.tile([128, 1024], F32, tag="z")
                    for j2 in range(2):
                        kc = 2 * h + j2
                        nc.tensor.matmul(
                            z_ps[:, j2 * 512:j2 * 512 + R2],
                            rpw[0:KTOT, par * DM + kc * 128:
                                par * DM + (kc + 1) * 128],
                            xaT[0:KTOT, off:off + R2],
                            start=True, stop=True,
                        )
                    zt = zsb.tile([128, 2 * R2], F16, tag="zt")
                    nc.scalar.activation(
                        out=zt.rearrange("p (t c) -> p t c", t=2),
                        in_=z_ps.rearrange("p (t c) -> p t c", t=2)[:, :, 0:R2],
                        func=AF.Tanh,
                    )
                    for j2 in range(2):
                        kc = 2 * h + j2
                        nc.tensor.matmul(
                            logits_ps[:, si * 512:si * 512 + R2],
                            uarep[:, kc * 128:(kc + 1) * 128],
                            zt[:, j2 * R2:(j2 + 1) * R2],
                            start=(kc == 0), stop=(kc == KC - 1),
                        )
                pair.append((TB, w, b0, off, R2, par))
                if len(pair) == 2:
                    flush_pair()
            flush_pair()

        # ---------------- final: normalize + output ----------------
        with ExitStack() as ctx:
            fps = ctx.enter_context(tc.tile_pool(name="fps", bufs=2, space="PSUM"))
            f2sb = ctx.enter_context(tc.tile_pool(name="f2sb", bufs=2))
            for blk in range(2):
                zc = f2sb.tile([128, 1], F32, tag="zc")
                nc.sync.dma_start(
                    out=zc, in_=xaesum[64:65, blk * 128:(blk + 1) * 128]
                )
                zc2 = f2sb.tile([128, 1], F32, tag="zc2")
                nc.vector.tensor_scalar(
                    out=zc2, in0=zc, scalar1=1e-30, scalar2=None, op0=ALU.add
                )
                nc.vector.reciprocal(out=recipz[:, blk:blk + 1], in_=zc2)
            for blk in range(2):
                out_ps = fps.tile([128, DM], F32, tag="op")
                nc.tensor.matmul(
                    out_ps, xaesum[0:65, blk * 128:(blk + 1) * 128], wb,
                    start=True, stop=True,
                )
                out_sb = f2sb.tile([128, DM], F32, tag="ob")
                nc.vector.tensor_scalar(
                    out=out_sb, in0=out_ps, scalar1=recipz[:, blk:blk + 1],
                    scalar2=None, op0=ALU.mult,
                )
                nc.sync.dma_start(
                    out=out.ap()[blk * 128:(blk + 1) * 128, :], in_=out_sb
                )

    nc.compile()
    return nc


def pack_inputs(x_others, x_mask):
    """Host-side layout prep: valid-first compaction + global sorted tiling."""
    mask_b = x_mask != 0
    c = mask_b.sum(1).astype(np.int64)
    order, plan, V = make_plan(c)
    # valid columns first within each batch (output is permutation-invariant)
    idx = np.argsort(~mask_b, axis=1, kind="stable")
    xm = (x_others.reshape(B, N, DOBJ).astype(np.float32)
          * x_mask[:, :, None].astype(np.float32)).astype(np.float16)
    xm_s = np.take_along_axis(xm, idx[:, :, None], axis=1)
    mk_s = np.take_along_axis(x_mask.astype(np.float16), idx, axis=1)
    # indicator value = query normalization 1/(sum(mask)+1e-5); a batch with
    # no valid entries has q_proj = 0 anyway, so any finite value works
    dsc = np.where(c > 0, 1.0 / (c + 1e-5), 1.0).astype(np.float16)

    cores = []
    for k in range(NCORES):
        xmp = np.zeros((V, DOBJ), np.float16)
        mkp = np.zeros((1, V), np.float16)
        indp = np.zeros((NIND, V), np.float16)
        bl = np.empty(BSH, np.int64)
        p = 0
        for (TB, w, b0, off) in plan:
            for j in range(TB):
                g = order[p + k * TB + j]
                s = off + j * w
                xmp[s:s + w] = xm_s[g, :w]
                mkp[0, s:s + w] = mk_s[g, :w]
                indp[j, s:s + w] = dsc[g]
                bl[b0 + j] = g
            p += 8 * TB
        cores.append((np.ascontiguousarray(xmp.T), mkp, indp, bl))
    return plan, V, cores


def _ensure_ntff_hook():
    """Provide antenv.axon_hooks if the image lacks it (NTFF profiling via
    ctypes into libaxon_pjrt.so), and stub out the artifact upload."""
    import types
    import ctypes
    import contextlib

    try:
        from antenv.axon_hooks import get_axon_ntff_profile_hook  # noqa: F401
    except ImportError:
        so_path = "/opt/axon/libaxon_pjrt.so"
        hook = None
        if os.path.exists(so_path):
            lib = ctypes.CDLL(so_path)
            if hasattr(lib, "axon_start_nrt_profile"):
                lib.axon_start_nrt_profile.argtypes = [
                    ctypes.POINTER(ctypes.c_int64), ctypes.c_size_t,
                ]
                lib.axon_start_nrt_profile.restype = ctypes.c_int64
                lib.axon_stop_nrt_profile.argtypes = [ctypes.c_char_p]
                lib.axon_stop_nrt_profile.restype = ctypes.c_int64

                @contextlib.contextmanager
                def _hook(output_dir, device_ids):
                    import jax
                    jax.devices()
                    if device_ids:
                        ids = (ctypes.c_int64 * len(device_ids))(*device_ids)
                        rc = lib.axon_start_nrt_profile(ids, len(device_ids))
                    else:
                        rc = lib.axon_start_nrt_profile(None, 0)
                    if rc != 0:
                        raise RuntimeError(f"axon_start_nrt_profile rc={rc}")
                    try:
                        yield
                    finally:
                        n = lib.axon_stop_nrt_profile(str(output_dir).encode())
                        print(f"ntff profile: {n} file(s) -> {output_dir}",
                              file=sys.stderr)

                hook = _hook

        import antenv
        mod = types.ModuleType("antenv.axon_hooks")
        mod.get_axon_ntff_profile_hook = lambda: hook
        mod.set_axon_ntff_profile_hook = lambda h: None
        sys.modules["antenv.axon_hooks"] = mod
        antenv.axon_hooks = mod

    import concourse.bass_utils as bu
    bu.upload_artifacts = lambda tmpdir: f"file://{tmpdir}"


def kernel(x_others, x_mask, conv_w, conv_b, Uq, Ur, Ua):
    x_others = np.asarray(x_others)
    x_mask = np.asarray(x_mask)
    conv_w = np.asarray(conv_w, dtype=np.float32)
    conv_b = np.asarray(conv_b, dtype=np.float32)
    Uq = np.asarray(Uq, dtype=np.float32)
    Ur = np.asarray(Ur, dtype=np.float32)
    Ua = np.asarray(Ua, dtype=np.float32)

    # host weight folding (tiny: 65x512 @ 512x512)
    wb32 = np.ascontiguousarray(
        np.concatenate([conv_w, conv_b.reshape(1, DM)], axis=0))   # [65, DM]
    wura16 = np.ascontiguousarray((wb32 @ Ur).astype(np.float16))
    wauq16 = np.ascontiguousarray((wb32 @ Uq).astype(np.float16))
    # lhsT for the logits matmul: within column block kc, column j holds
    # Ua[kc*128 + k] at partition k (replicated across the 128 out columns)
    uarep16 = np.empty((128, DM), np.float16)
    ua16 = Ua.reshape(DM).astype(np.float16)
    for kc in range(KC):
        uarep16[:, kc * 128:(kc + 1) * 128] = ua16[kc * 128:(kc + 1) * 128, None]

    plan, V, cores = pack_inputs(x_others, x_mask)
    nc = build_nc(plan, V)

    in_maps = []
    for k in range(NCORES):
        xmt_k, mkp_k, indp_k, _ = cores[k]
        in_maps.append({
            "xmt": xmt_k,
            "mask": mkp_k,
            "ind8": indp_k,
            "wb32": wb32,
            "wura16": wura16,
            "wauq16": wauq16,
            "uarep16": uarep16,
        })

    from concourse.bass_utils import run_bass_kernel_spmd

    trace = os.environ.get("KERNEL_TRACE", "0") == "1"
    if trace:
        _ensure_ntff_hook()
    tmpdir = None
    if trace:
        import tempfile
        os.makedirs("/root/problem/traces", exist_ok=True)
        tmpdir = tempfile.mkdtemp(dir="/root/problem/traces")
        print(f"trace dir: {tmpdir}", file=sys.stderr)
    res = run_bass_kernel_spmd(
        nc, in_maps, core_ids=list(range(NCORES)), trace=trace, tmpdir=tmpdir
    )
    if trace and res.exec_time_ns is not None:
        print(f"HW exec time: {res.exec_time_ns} ns", file=sys.stderr)
        kernel.last_exec_time_ns = res.exec_time_ns
        kernel.last_trace = res.instructions_and_trace
    out = np.empty((B, DM), dtype=np.float32)
    for k, r in enumerate(res.results):
        out[cores[k][3]] = r["out"]
    return out


if __name__ == "__main__":
    rng = np.random.default_rng(0)
    x = rng.standard_normal((B, N * DOBJ), dtype=np.float32)
    mask = rng.integers(0, 2, (B, N)).astype(np.float32)
    w = rng.standard_normal((DOBJ, DM), dtype=np.float32) / 8.0
    cbv = np.zeros((DM,), dtype=np.float32)
    uq = rng.standard_normal((DM, DM), dtype=np.float32) / 22.6
    urm = rng.standard_normal((DM, DM), dtype=np.float32) / 22.6
    uav = rng.standard_normal((DM, ), dtype=np.float32) * 0.1
    out = kernel(x, mask, w, cbv, uq, urm, uav)
    print(out.shape, out.dtype)
